# revision 9
# baseline (speedup 1.0000x reference)
"""Trainium2 Bass kernel for nn_BaseCamera_1589137899573.

Computes PSF of a phase-mask camera:
  field = aperture * exp(i*(const_phase + spline_bias))   (4096^2, nonzero on central 2048^2)
  psf   = |IFFT2( FFT2(field) * Hs )|^2                   (Hs = ifftshift(exp(i*H_phase)))
  out   = crop 728x728, normalize by sum.

Distribution over 8 NeuronCores (v2 — fp16 datapath, DMA corner turns):
  P1: band rows (2048) split 256/core; phase -> field via Sin activation
      (per-partition pi/2 bias selects cos rows), row-FFT as radix-64
      two-stage matmul DFT in fp16.  Corner turn between the two DFT
      stages goes through a DRAM bounce (2 DMAs) instead of PE transposes.
  A2A: AllToAll row-spectra (fp16) -> each core holds 512 spectral cols.
  P2: per column-chunk: col-FFT stage A -> DMA turn -> stage B + H-mult ->
      col-IFFT stage A (rows pruned to the 768-row crop band) -> DMA turn ->
      pruned stage B; writes [k, r]-major planes.
  A2A2 + P3: row-IFFT for 96 of the 768 band rows per core, |.|^2.
  Host: assemble, crop to 728^2, normalize.

Scaling: WB (fwd stage B) x 1/64 per use, WAI (inv stage A) x 1/64 per use;
product over fwd+inv = 1/4096^2-equivalent; final sum-normalization makes
any residual global scale irrelevant.  All intermediates stay well inside
fp16 range (validated offline: rel err ~6e-4 vs f64 reference).
"""

import numpy as np

# ---------------- problem constants (hardcoded; must match reference) -------
N = 4096              # WAVE_RES
V = 2048              # VALID_RES (band size)
B0 = 1024             # band start (pad)
PITCH = 2e-6
SENSOR_D = N * PITCH
D1 = 0.05
D2 = 0.05
FOCAL = D1 * D2 / (D1 + D2)
WCROP = 728
LAM = 5.32e-7
UP = 2
TWO_PI = 2.0 * np.pi
K_WAVE = TWO_PI / LAM

CROP_S = N // 2 - WCROP // 2 + 1          # 1685
RHI_LO, RHI_HI = CROP_S // 64, (CROP_S + WCROP - 1) // 64   # 26, 37
NSEL = RHI_HI - RHI_LO + 1                # 12 selected high-digit values
BAND_LO = 64 * RHI_LO                     # 1664
BAND_W = 64 * NSEL                        # 768
CROP_OFF = CROP_S - BAND_LO               # 21

NC = 8                # cores
RPC = V // NC         # 256 band rows per core in P1
CPC = N // NC         # 512 spectral cols per core in P2
KCHUNK = 128          # P2 k_c chunk
NCHUNK = CPC // KCHUNK  # 4
RPC3 = BAND_W // NC   # 96 rows per core in P3
NPEN1 = 128           # P1 half size (pencils)

F32 = np.float32
F16 = np.float16


# ---------------- small host helpers ----------------------------------------
def _thomas(r):
    """diag=4 off-diag=1 tridiagonal solve, float32 to mirror reference."""
    n = r.shape[0]
    cp = np.zeros(n, np.float32)
    dp = np.zeros(n, np.float32)
    c_prev = np.float32(0.0)
    d_prev = np.float32(0.0)
    for i in range(n):
        den = np.float32(4.0) - c_prev
        c_prev = np.float32(1.0) / den
        d_prev = (r[i] - d_prev) / den
        cp[i] = c_prev
        dp[i] = d_prev
    x = np.zeros(n, np.float32)
    x_next = np.float32(0.0)
    for i in range(n - 1, -1, -1):
        x_next = dp[i] - cp[i] * x_next
        x[i] = x_next
    return x


def spline_quadrant(optim_param):
    """q[i,j] = natural-cubic-spline(mp_log) at r=sqrt((i+.5)^2+(j+.5)^2), [1024,1024]."""
    p = np.asarray(optim_param, np.float32)
    mp = np.repeat(p, UP)
    y = np.concatenate([mp, np.zeros(V // 2, np.float32)])       # len 2048
    n = y.shape[0]
    rhs = (6.0 * (y[2:].astype(np.float64) - 2.0 * y[1:-1] + y[:-2])).astype(np.float32)
    M = np.concatenate([np.zeros(1, np.float32), _thomas(rhs), np.zeros(1, np.float32)])
    half = V // 2
    coord = np.arange(half, dtype=np.float32) + 0.5
    r = np.sqrt(coord[:, None] ** 2 + coord[None, :] ** 2)
    ind = np.clip(np.floor(r).astype(np.int64), 0, n - 2)
    t = r - ind.astype(np.float32)
    y0, y1 = y[ind], y[ind + 1]
    m0, m1 = M[ind], M[ind + 1]
    b = (y1 - y0) - (2.0 * m0 + m1) / 6.0
    return y0 + t * (b + t * (m0 / 2.0 + t * (m1 - m0) / 6.0))


def bias_band(optim_param):
    """Full mirrored bias map on the 2048^2 band."""
    q = spline_quadrant(optim_param)
    row = np.concatenate([q[:, ::-1], q], axis=1)
    return np.concatenate([row[::-1, :], row], axis=0)          # [2048, 2048]


def const_phase_band():
    """(input_phase + lens_phase) on the 2048^2 band, f64."""
    coords = (PITCH * (np.arange(N, dtype=np.float32) - N // 2)).astype(np.float32)
    cb = coords[B0:B0 + V].astype(np.float64)
    r2 = cb[:, None] ** 2 + cb[None, :] ** 2
    return np.float64(K_WAVE) * r2 * (1.0 / (2 * D1) - 1.0 / (2 * FOCAL))


def h_spec_planes():
    """ifftshifted transfer function exp(i*H_phase): (re, im) [4096,4096] f64."""
    fx = ((np.arange(1, N + 1, dtype=np.float32) - np.float32(N / 2)) / np.float32(SENSOR_D)).astype(np.float32)
    FY, FX = np.meshgrid(fx, fx, indexing="ij")
    arg = np.maximum((np.float32(1.0 / LAM)) ** 2 - FX.astype(np.float64) ** 2 - FY.astype(np.float64) ** 2, 0.0)
    w1 = np.sqrt(arg).astype(np.float32)
    hp = (np.float32(TWO_PI) * w1 * np.float32(D2)).astype(np.float32).astype(np.float64)
    hre = np.cos(hp)
    him = np.sin(hp)
    hre = np.fft.ifftshift(hre)
    him = np.fft.ifftshift(him)
    return hre, him


# ---------------- DFT stage matrices (complex->real 2x blocks) ---------------
def _c2r_lhsT(E):
    """Complex matrix E [out m, in k] -> real lhsT [2k, 2m] for out=lhsT.T@rhs."""
    m, k = E.shape
    W = np.zeros((2 * k, 2 * m), np.float64)
    W[:k, :m] = E.real.T
    W[k:, :m] = -E.imag.T
    W[:k, m:] = E.imag.T
    W[k:, m:] = E.real.T
    return W


def stage_a_fwd_mats():
    """WA[c_lo]: [64, 128]; contracts c_hi' (32 band-high-digits), out k_lo."""
    klo = np.arange(64)[:, None]
    chi = np.arange(32)[None, :]
    mats = []
    for c_lo in range(64):
        E = np.exp(-2j * np.pi * ((16 + chi) * klo % 64) / 64.0) \
            * np.exp(-2j * np.pi * (c_lo * klo) / 4096.0)
        mats.append(_c2r_lhsT(E))
    return np.stack(mats).astype(F16)                            # [64, 64, 128]


def stage_b_fwd_mat():
    """WB: [128, 128]; contracts c_lo (64), out k_hi. DFT-64 scaled 1/64."""
    khi = np.arange(64)[:, None]
    clo = np.arange(64)[None, :]
    E = np.exp(-2j * np.pi * (clo * khi % 64) / 64.0) / 64.0
    return _c2r_lhsT(E).astype(F16)                              # [128, 128]


def stage_a_inv_mats(scale):
    """WAI[m_lo]: [128, 128]; contracts m_hi (full 64), out r_lo, +sign, *scale."""
    rlo = np.arange(64)[:, None]
    mhi = np.arange(64)[None, :]
    mats = []
    for m_lo in range(64):
        E = np.exp(2j * np.pi * (mhi * rlo % 64) / 64.0) \
            * np.exp(2j * np.pi * (m_lo * rlo) / 4096.0) * scale
        mats.append(_c2r_lhsT(E))
    return np.stack(mats).astype(F16)                            # [64, 128, 128]


def stage_b_inv_mat():
    """WBI: [128, 64]; contracts m_lo, out r_hi in {26..37}.
    Output partitions: re at 0:12, im at 32:44."""
    rhi = np.arange(RHI_LO, RHI_HI + 1)[:, None]
    mlo = np.arange(64)[None, :]
    E = np.exp(2j * np.pi * (mlo * rhi % 64) / 64.0)
    W = _c2r_lhsT(E)                                             # [128, 24]
    out = np.zeros((128, 64), np.float64)
    out[:, 0:12] = W[:, 0:12]
    out[:, 32:44] = W[:, 12:24]
    return out.astype(F16)


# ============================================================================
#                        host-side per-core input builders
# ============================================================================
def _syn_layout(plane, rs):
    """[2048, 2048] f32 -> [128, 32*RPC]: p = 64u + 32*is + c_hi',
    f = v*RPC + r''  (c' = 32u + v + 64*c_hi').  Duplicated on both is rows."""
    out = np.empty((128, 32, RPC), F32)
    x = plane[rs].reshape(RPC, 32, 2, 32)      # r'', c_hi', u, v
    for u in range(2):
        out[64 * u:64 * u + 32] = x[:, :, u, :].transpose(1, 2, 0)
        out[64 * u + 32:64 * u + 64] = x[:, :, u, :].transpose(1, 2, 0)
    return np.ascontiguousarray(out.reshape(128, 32 * RPC))


def _h_layout(hre, him):
    """[4096 m, 512 k_c] -> [128, 32768] fp16: p = m_hi (re) / 64+m_hi (im),
    f = chunk*(64*KCHUNK) + m_lo*KCHUNK + k_cc."""
    out = np.empty((128, NCHUNK, 64, KCHUNK), F16)
    t = hre.reshape(64, 64, NCHUNK, KCHUNK)    # m_hi, m_lo, chunk, k_cc
    b = him.reshape(64, 64, NCHUNK, KCHUNK)
    out[:64] = t.transpose(0, 2, 1, 3)
    out[64:] = b.transpose(0, 2, 1, 3)
    return np.ascontiguousarray(out.reshape(128, NCHUNK * 64 * KCHUNK))


_CONST_CACHE = {}


def _shared_consts():
    if "c" not in _CONST_CACHE:
        cph = const_phase_band()
        hre, him = h_spec_planes()
        WA = stage_a_fwd_mats()                    # [64, 64, 128] fp16
        WA2 = np.concatenate([WA, WA], axis=1)     # [64, 128, 128]
        _CONST_CACHE["c"] = dict(
            cph=cph, hre=hre.astype(F16), him=him.astype(F16),
            wa=np.ascontiguousarray(WA2),
            wb=stage_b_fwd_mat(),
            wai=stage_a_inv_mats(1.0 / 64.0),
            wbi=stage_b_inv_mat(),
        )
    return _CONST_CACHE["c"]


def build_core_inputs(optim_param):
    C = _shared_consts()
    bias = bias_band(optim_param).astype(np.float64)
    # centered in [-pi, pi) so Sin-activation args stay within +-3pi/2
    ph = (np.mod(C["cph"] + bias + np.pi, TWO_PI) - np.pi).astype(F32)
    in_maps = []
    for c in range(NC):
        rs = slice(c * RPC, (c + 1) * RPC)
        ks = slice(c * CPC, (c + 1) * CPC)
        hre = C["hre"][:, ks].astype(F32)
        him = C["him"][:, ks].astype(F32)
        in_maps.append({
            "wa": C["wa"], "wb": C["wb"], "wai": C["wai"], "wbi": C["wbi"],
            "ph": _syn_layout(ph, rs),
            "h1": _h_layout(hre, him),
            "h2": _h_layout(-him, hre),
        })
    return in_maps


# ============================================================================
#                              bass / tile program
# ============================================================================
_NC_CACHE = {}


def _build_nc():
    import concourse.bacc as bacc
    import concourse.mybir as mybir
    import concourse.tile as tile

    dt = mybir.dt.float32
    dh = mybir.dt.float16
    nc = bacc.Bacc("TRN2", target_bir_lowering=False, debug=False,
                   num_devices=NC)

    D = {}
    def din(name, shape, dtype):
        D[name] = nc.dram_tensor(name, list(shape), dtype,
                                 kind="ExternalInput").ap()
    din("wa", (64, 128, 128), dh); din("wb", (128, 128), dh)
    din("wai", (64, 128, 128), dh); din("wbi", (128, 64), dh)
    din("ph", (128, 32 * RPC), dt)
    din("h1", (128, NCHUNK * 64 * KCHUNK), dh)
    din("h2", (128, NCHUNK * 64 * KCHUNK), dh)
    for nm, shape in (
        ("a1i_re", [V, CPC]), ("a1i_im", [V, CPC]),
        ("a1o_re", [V, CPC]), ("a1o_im", [V, CPC]),
        ("a2i_re", [N, RPC3]), ("a2i_im", [N, RPC3]),
        ("a2o_re", [N, RPC3]), ("a2o_im", [N, RPC3]),
        ("sc10", [128, 64 * NPEN1]), ("sc11", [128, 64 * NPEN1]),
        ("scf0", [128, 64 * KCHUNK]), ("scf1", [128, 64 * KCHUNK]),
        ("scf2", [128, 64 * KCHUNK]), ("scf3", [128, 64 * KCHUNK]),
        ("sci0", [128, 64 * KCHUNK]), ("sci1", [128, 64 * KCHUNK]),
        ("sci2", [128, 64 * KCHUNK]), ("sci3", [128, 64 * KCHUNK]),
        ("sc3", [128, 64 * RPC3]),
    ):
        D[nm] = nc.dram_tensor(nm, shape, dh).ap()
    D["out"] = nc.dram_tensor("out", [RPC3, BAND_W], dt, kind="ExternalOutput").ap()

    with tile.TileContext(nc) as tc:
        _emit(nc, tc, mybir, D)
    nc.compile()
    return nc


def _emit(nc, tc, mybir, D):
    dt = mybir.dt.float32
    dh = mybir.dt.float16
    AF = mybir.ActivationFunctionType
    HALF_PI = float(np.pi / 2)
    tgl = [0]

    def drain(dst, src):
        # gpsimd cannot access PSUM; rotate vector/scalar only
        tgl[0] ^= 1
        if tgl[0]:
            nc.vector.tensor_copy(dst, src)
        else:
            nc.scalar.copy(dst, src)

    with (
        tc.tile_pool(name="c1", bufs=1) as c1,
        tc.tile_pool(name="big", bufs=4) as bpool,
        tc.tile_pool(name="ps", bufs=8, space="PSUM") as ppool,
    ):
        # ---- constants resident all phases ----
        wa_t = c1.tile([128, 64 * 128], dh, tag="wa")
        nc.sync.dma_start(wa_t[:].rearrange("k (m n) -> k m n", n=128),
                          D["wa"].rearrange("m k n -> k m n"))
        wb_t = c1.tile([128, 128], dh, tag="wb")
        nc.sync.dma_start(wb_t[:], D["wb"])
        bvec = c1.tile([128, 1], dt, tag="bv")
        nc.vector.memset(bvec[0:32], HALF_PI)
        nc.vector.memset(bvec[32:64], 0.0)
        nc.vector.memset(bvec[64:96], HALF_PI)
        nc.vector.memset(bvec[96:128], 0.0)

        def WA(j, base):
            return wa_t[base:base + 64, j * 128:(j + 1) * 128]

        # ============================ P1 ============================
        with tc.tile_pool(name="syn", bufs=2) as spool:
            for half in range(2):
                rsl = slice(half * NPEN1, (half + 1) * NPEN1)
                pht = spool.tile([128, 32 * NPEN1], dt, tag="ph")
                nc.sync.dma_start(
                    pht[:].rearrange("p (v r) -> p v r", r=NPEN1),
                    D["ph"].rearrange("p (v r) -> p v r", r=RPC)[:, :, rsl])
                Ft = spool.tile([128, 32 * NPEN1], dh, tag="F")
                nc.scalar.activation(Ft[:], pht[:], AF.Sin, bias=bvec[:])
                Fv = Ft[:].rearrange("p (v r) -> p v r", r=NPEN1)
                # stage A: 64 matmuls N=128, j-major dense drains
                t1 = bpool.tile([128, 64 * NPEN1], dh, tag="big")
                for g in range(16):
                    ps = ppool.tile([128, 512], dt, tag="ps")
                    for q in range(4):
                        j = 4 * g + q
                        u, v = divmod(j, 32)
                        nc.tensor.matmul(ps[:, q * NPEN1:(q + 1) * NPEN1],
                                         WA(j, 64 * u), Fv[64 * u:64 * u + 64, v, :],
                                         start=True, stop=True)
                    drain(t1[:, g * 512:(g + 1) * 512], ps[:])
                # corner turn via DRAM bounce: t1[p=klo+64is, f=clo*128+pen]
                #   -> t2[p=clo+64is, f=klo*128+pen]
                sc = D["sc10"] if half == 0 else D["sc11"]
                nc.gpsimd.dma_start(sc, t1[:])
                t2 = bpool.tile([128, 64 * NPEN1], dh, tag="big")
                scv = sc.rearrange("(i k) (c n) -> i c k n", i=2, n=NPEN1)
                for isim in range(2):
                    nc.sync.dma_start(
                        t2[64 * isim:64 * isim + 64, :]
                        .rearrange("p (k n) -> p k n", n=NPEN1),
                        scv[isim])
                # stage B: rhs viewed pen-major so t3 matches a1i layout
                t2v = t2[:].rearrange("p (k n) -> p n k", n=NPEN1)
                t3 = bpool.tile([128, 64 * NPEN1], dh, tag="big")
                for m in range(16):
                    ps = ppool.tile([128, 512], dt, tag="ps")
                    nc.tensor.matmul(ps[:], wb_t[:], t2v[:, m * 8:(m + 1) * 8, :],
                                     start=True, stop=True)
                    drain(t3[:, m * 512:(m + 1) * 512], ps[:])
                # out: t3 [p=khi+64is, f=pen*64+klo] -> a1i [(s*256+r)*512 + khl*64+klo]
                for isim, nm in ((0, "a1i_re"), (1, "a1i_im")):
                    dvw = D[nm].rearrange("(s r) (khl kl) -> s khl r kl", s=8, khl=8)
                    for s in range(8):
                        eng = nc.sync if s % 2 == 0 else nc.gpsimd
                        eng.dma_start(
                            dvw[s, :, rsl, :],
                            t3[64 * isim + 8 * s:64 * isim + 8 * s + 8, :]
                            .rearrange("p (pen kl) -> p pen kl", kl=64))

        # ============================ A2A 1 ============================
        rg = [list(range(NC))]
        nc.gpsimd.collective_compute("AllToAll", mybir.AluOpType.bypass,
                                     ins=[D["a1i_re"]], outs=[D["a1o_re"]],
                                     replica_groups=rg)
        nc.gpsimd.collective_compute("AllToAll", mybir.AluOpType.bypass,
                                     ins=[D["a1i_im"]], outs=[D["a1o_im"]],
                                     replica_groups=rg)

        # ============================ P2 ============================
        with (
            tc.tile_pool(name="c2", bufs=1) as c2,
            tc.tile_pool(name="lp", bufs=2) as lpool,
            tc.tile_pool(name="hp", bufs=3) as hpool,
            tc.tile_pool(name="kp", bufs=2) as kpool,
        ):
            wai_t = c2.tile([128, 64 * 128], dh, tag="wai")
            nc.sync.dma_start(wai_t[:].rearrange("k (m n) -> k m n", n=128),
                              D["wai"].rearrange("m k n -> k m n"))
            wbi_t = c2.tile([128, 64], dh, tag="wbi")
            nc.sync.dma_start(wbi_t[:], D["wbi"])

            def WAI(j):
                return wai_t[:, j * 128:(j + 1) * 128]

            SCF = (D["scf0"], D["scf1"], D["scf2"], D["scf3"])
            SCI = (D["sci0"], D["sci1"], D["sci2"], D["sci3"])
            for chunk in range(NCHUNK):
                L = lpool.tile([128, 32 * KCHUNK], dh, tag="L")
                for isim, nm in ((0, "a1o_re"), (1, "a1o_im")):
                    av = D[nm].rearrange("(rh u v) k -> u rh v k", u=2, v=32)
                    for u in range(2):
                        nc.sync.dma_start(
                            L[64 * u + 32 * isim:64 * u + 32 * isim + 32, :]
                            .rearrange("p (v kc) -> p v kc", kc=KCHUNK),
                            av[u, :, :, chunk * KCHUNK:(chunk + 1) * KCHUNK])
                # col-FFT stage A (mats by c_lo = 32u+v), j-major drains
                t1 = bpool.tile([128, 64 * KCHUNK], dh, tag="big")
                Lv = L[:].rearrange("p (v kc) -> p v kc", kc=KCHUNK)
                for g in range(16):
                    ps = ppool.tile([128, 512], dt, tag="ps")
                    for q in range(4):
                        j = 4 * g + q
                        u, v = divmod(j, 32)
                        nc.tensor.matmul(ps[:, q * KCHUNK:(q + 1) * KCHUNK],
                                         WA(j, 64 * u),
                                         Lv[64 * u:64 * u + 64, v, :],
                                         start=True, stop=True)
                    drain(t1[:, g * 512:(g + 1) * 512], ps[:])
                # turn: t1[p=mlo+64is, f=clo*K+kcc] -> t2[p=clo+64is, f=mlo*K+kcc]
                nc.gpsimd.dma_start(SCF[chunk], t1[:])
                t2 = bpool.tile([128, 64 * KCHUNK], dh, tag="big")
                scv = SCF[chunk].rearrange("(i m) (c k) -> i c m k", i=2, k=KCHUNK)
                for isim in range(2):
                    nc.sync.dma_start(
                        t2[64 * isim:64 * isim + 64, :]
                        .rearrange("p (m k) -> p m k", k=KCHUNK),
                        scv[isim])
                # col-FFT stage B + H-mult -> SH [p=m_hi+64is, f=m_lo*K+kcc]
                # h1 = (hre | him), h2 = (-him | hre); psum operands are
                # exempt from the same-start-partition rule, SBUF pairs align.
                SH = bpool.tile([128, 64 * KCHUNK], dh, tag="big")
                nbank = (64 * KCHUNK) // 512
                for mm in range(nbank):
                    ht1 = hpool.tile([128, 512], dh, tag="h1")
                    ht2 = hpool.tile([128, 512], dh, tag="h2")
                    off = chunk * 64 * KCHUNK + mm * 512
                    nc.sync.dma_start(ht1[:], D["h1"][:, off:off + 512])
                    nc.sync.dma_start(ht2[:], D["h2"][:, off:off + 512])
                    ps = ppool.tile([128, 512], dt, tag="ps")
                    nc.tensor.matmul(ps[:], wb_t[:],
                                     t2[:, mm * 512:(mm + 1) * 512],
                                     start=True, stop=True)
                    sl = slice(mm * 512, (mm + 1) * 512)
                    ta = kpool.tile([128, 512], dh, tag="ta")
                    tb = kpool.tile([128, 512], dh, tag="tb")
                    nc.vector.tensor_mul(ta[0:64], ps[0:64], ht1[0:64])
                    nc.vector.tensor_mul(ta[64:128], ps[0:64], ht1[64:128])
                    nc.vector.tensor_mul(tb[0:64], ps[64:128], ht2[0:64])
                    nc.vector.tensor_mul(tb[64:128], ps[64:128], ht2[64:128])
                    nc.vector.tensor_add(SH[0:64, sl], ta[0:64], tb[0:64])
                    nc.gpsimd.tensor_add(SH[64:128, sl], ta[64:128], tb[64:128])
                # col-IFFT stage A (mats by m_lo), out digit r_lo; j-major drains
                ti = bpool.tile([128, 64 * KCHUNK], dh, tag="big")
                for g in range(16):
                    ps = ppool.tile([128, 512], dt, tag="ps")
                    for q in range(4):
                        j = 4 * g + q
                        nc.tensor.matmul(ps[:, q * KCHUNK:(q + 1) * KCHUNK],
                                         WAI(j),
                                         SH[:, j * KCHUNK:(j + 1) * KCHUNK],
                                         start=True, stop=True)
                    drain(ti[:, g * 512:(g + 1) * 512], ps[:])
                # turn: ti[p=rlo+64is, f=mlo*K+kcc] -> tj[p=mlo+64is, f=rlo*K+kcc]
                nc.gpsimd.dma_start(SCI[chunk], ti[:])
                tj = bpool.tile([128, 64 * KCHUNK], dh, tag="big")
                scv = SCI[chunk].rearrange("(i r) (m k) -> i m r k", i=2, k=KCHUNK)
                for isim in range(2):
                    nc.sync.dma_start(
                        tj[64 * isim:64 * isim + 64, :]
                        .rearrange("p (r k) -> p r k", k=KCHUNK),
                        scv[isim])
                # col-IFFT stage B (pruned to 24 rows); rhs viewed kcc-major
                tjv = tj[:].rearrange("p (r k) -> p k r", k=KCHUNK)
                tk = bpool.tile([64, 64 * KCHUNK], dh, tag="big")
                for m in range(nbank):
                    ps = ppool.tile([128, 512], dt, tag="ps")
                    nc.tensor.matmul(ps[0:64, :], wbi_t[:],
                                     tjv[:, m * 8:(m + 1) * 8, :],
                                     start=True, stop=True)
                    drain(tk[0:12, m * 512:(m + 1) * 512], ps[0:12, :])
                    drain(tk[32:44, m * 512:(m + 1) * 512], ps[32:44, :])
                # out: tk [p=idx+32isim, f=kcc*64+rl] -> a2i [(s*512+kc)*96 + idx*8+rl3]
                for isim, nm in ((0, "a2i_re"), (1, "a2i_im")):
                    dvw = D[nm].rearrange("(s kc) (idx rl3) -> s idx kc rl3",
                                          s=8, idx=12)
                    tv = tk[32 * isim:32 * isim + 12, :].rearrange(
                        "p (kc s rl3) -> p kc s rl3", s=8, rl3=8)
                    for s in range(8):
                        eng = nc.sync if s % 2 == 0 else nc.gpsimd
                        eng.dma_start(
                            dvw[s, :, chunk * KCHUNK:(chunk + 1) * KCHUNK, :],
                            tv[:, :, s, :])

        # ============================ A2A 2 ============================
        nc.gpsimd.collective_compute("AllToAll", mybir.AluOpType.bypass,
                                     ins=[D["a2i_re"]], outs=[D["a2o_re"]],
                                     replica_groups=rg)
        nc.gpsimd.collective_compute("AllToAll", mybir.AluOpType.bypass,
                                     ins=[D["a2i_im"]], outs=[D["a2o_im"]],
                                     replica_groups=rg)

        # ============================ P3 ============================
        with (
            tc.tile_pool(name="c2b", bufs=1) as c2b,
            tc.tile_pool(name="lp3", bufs=1) as lp3,
            tc.tile_pool(name="to3", bufs=2) as to3,
        ):
            wai_t = c2b.tile([128, 64 * 128], dh, tag="wai3")
            nc.sync.dma_start(wai_t[:].rearrange("k (m n) -> k m n", n=128),
                              D["wai"].rearrange("m k n -> k m n"))
            wbi_t = c2b.tile([128, 64], dh, tag="wbi3")
            nc.sync.dma_start(wbi_t[:], D["wbi"])
            LB = lp3.tile([128, 64 * RPC3], dh, tag="LB")
            for isim, nm in ((0, "a2o_re"), (1, "a2o_im")):
                nc.sync.dma_start(
                    LB[64 * isim:64 * isim + 64, :]
                    .rearrange("p (kl r) -> p kl r", r=RPC3),
                    D[nm].rearrange("(kh kl) r -> kh kl r", kl=64))
            # stage A: contract k_hi, mats by k_lo, out digit c_lo; j-major drains
            t1 = bpool.tile([128, 64 * RPC3], dh, tag="big")
            for g in range(13):
                ps = ppool.tile([128, 512], dt, tag="ps")
                qn = min(5, 64 - 5 * g)
                for q in range(qn):
                    j = 5 * g + q
                    nc.tensor.matmul(ps[:, q * RPC3:(q + 1) * RPC3],
                                     wai_t[:, j * 128:(j + 1) * 128],
                                     LB[:, j * RPC3:(j + 1) * RPC3],
                                     start=True, stop=True)
                drain(t1[:, 5 * g * RPC3:(5 * g + qn) * RPC3],
                      ps[:, 0:qn * RPC3])
            # turn: t1[p=rlo+64is, f=klo*96+r] -> t2[p=klo+64is, f=rlo*96+r]
            nc.gpsimd.dma_start(D["sc3"], t1[:])
            t2 = bpool.tile([128, 64 * RPC3], dh, tag="big")
            scv = D["sc3"].rearrange("(i c) (k r) -> i k c r", i=2, r=RPC3)
            for isim in range(2):
                nc.sync.dma_start(
                    t2[64 * isim:64 * isim + 64, :]
                    .rearrange("p (c r) -> p c r", r=RPC3),
                    scv[isim])
            # stage B pruned + |.|^2 ; rhs viewed r-major
            t2v = t2[:].rearrange("p (c r) -> p r c", r=RPC3)
            tout = bpool.tile([12, 64 * RPC3], dt, tag="big")
            for m in range((64 * RPC3) // 512):
                ps = ppool.tile([128, 512], dt, tag="ps")
                nc.tensor.matmul(ps[0:64, :], wbi_t[:],
                                 t2v[:, m * 8:(m + 1) * 8, :],
                                 start=True, stop=True)
                sq1 = to3.tile([12, 512], dt, tag="sq1")
                sq2 = to3.tile([12, 512], dt, tag="sq2")
                nc.scalar.activation(sq1[:], ps[0:12, :], AF.Square)
                nc.scalar.activation(sq2[:], ps[32:44, :], AF.Square)
                nc.vector.tensor_add(tout[:, m * 512:(m + 1) * 512],
                                     sq1[:], sq2[:])
            nc.sync.dma_start(
                D["out"].rearrange("r (ci cl) -> ci r cl", ci=12),
                tout[:].rearrange("p (r cl) -> p r cl", cl=64))


# ============================================================================
#                                   kernel()
# ============================================================================
def kernel(optim_param, _trace=False):
    from concourse.bass_utils import run_bass_kernel_spmd
    if "nc" not in _NC_CACHE:
        _NC_CACHE["nc"] = _build_nc()
    nc = _NC_CACHE["nc"]
    in_maps = build_core_inputs(np.asarray(optim_param, F32))
    res = run_bass_kernel_spmd(nc, in_maps, list(range(NC)), trace=_trace)
    outs = [res.results[c]["out"] for c in range(NC)]      # [96, 768] each
    band = np.empty((BAND_W, BAND_W), np.float64)
    for c in range(NC):
        o = np.asarray(outs[c], np.float64)                # rows idx*8+rl3
        for idx in range(NSEL):
            band[64 * idx + 8 * c:64 * idx + 8 * c + 8, :] = o[8 * idx:8 * idx + 8, :]
    region = band[CROP_OFF:CROP_OFF + WCROP, CROP_OFF:CROP_OFF + WCROP]
    out = (region / region.sum()).astype(F32)[None, None]
    if _trace:
        return out, res
    return out


# revision 15
# speedup vs baseline: 1.1232x; 1.1232x over previous
"""Trainium2 Bass kernel for nn_BaseCamera_1589137899573.

Computes PSF of a phase-mask camera:
  field = aperture * exp(i*(const_phase + spline_bias))   (4096^2, nonzero on central 2048^2)
  psf   = |IFFT2( FFT2(field) * Hs )|^2                   (Hs = ifftshift(exp(i*H_phase)))
  out   = crop 728x728, normalize by sum.

Distribution over 8 NeuronCores (v3 — fp16 datapath, DMA corner turns,
software-pipelined column chunks):
  P1: band rows (2048) split 256/core; phase -> field via a single Sin
      activation (per-partition pi/2 bias selects cos rows); row-FFT as
      radix-64 two-stage matmul DFT in fp16.  The corner turn between the
      DFT stages goes through a DRAM bounce (2 DMAs) instead of PE
      transposes.
  A2A: AllToAll row-spectra (fp16) -> each core holds 512 spectral cols.
  P2: per column-chunk (pipelined, chunk c+1's front half overlaps chunk
      c's back half): col-FFT stage A -> DMA turn -> stage B; H-multiply
      produces TA = S_re*(hre|him), TB = S_im*(-him|hre) and the column
      IFFT stage A contracts TA+TB via PSUM accumulation (no explicit
      add); rows pruned to the 768-row crop band -> DMA turn -> pruned
      stage B; writes [k, r]-major planes.
  A2A2 + P3: row-IFFT for 96 of the 768 band rows per core, |.|^2.
  Host: assemble, crop to 728^2, normalize.

Scaling: WB x 1/64 per use, WAI x 1/64 per use; the final
sum-normalization makes any residual global scale irrelevant.  All
intermediates stay well inside fp16 range (validated offline, ~6e-4).
"""

import numpy as np

# ---------------- problem constants (hardcoded; must match reference) -------
N = 4096              # WAVE_RES
V = 2048              # VALID_RES (band size)
B0 = 1024             # band start (pad)
PITCH = 2e-6
SENSOR_D = N * PITCH
D1 = 0.05
D2 = 0.05
FOCAL = D1 * D2 / (D1 + D2)
WCROP = 728
LAM = 5.32e-7
UP = 2
TWO_PI = 2.0 * np.pi
K_WAVE = TWO_PI / LAM

CROP_S = N // 2 - WCROP // 2 + 1          # 1685
RHI_LO, RHI_HI = CROP_S // 64, (CROP_S + WCROP - 1) // 64   # 26, 37
NSEL = RHI_HI - RHI_LO + 1                # 12 selected high-digit values
BAND_LO = 64 * RHI_LO                     # 1664
BAND_W = 64 * NSEL                        # 768
CROP_OFF = CROP_S - BAND_LO               # 21

NC = 8                # cores
RPC = V // NC         # 256 band rows per core in P1
CPC = N // NC         # 512 spectral cols per core in P2
KCHUNK = 128          # P2 k_c chunk
NCHUNK = CPC // KCHUNK  # 4
RPC3 = BAND_W // NC   # 96 rows per core in P3
NPEN1 = 128           # P1 half size (pencils)

F32 = np.float32
F16 = np.float16


# ---------------- small host helpers ----------------------------------------
def _thomas(r):
    """diag=4 off-diag=1 tridiagonal solve, float32 to mirror reference."""
    n = r.shape[0]
    cp = np.zeros(n, np.float32)
    dp = np.zeros(n, np.float32)
    c_prev = np.float32(0.0)
    d_prev = np.float32(0.0)
    for i in range(n):
        den = np.float32(4.0) - c_prev
        c_prev = np.float32(1.0) / den
        d_prev = (r[i] - d_prev) / den
        cp[i] = c_prev
        dp[i] = d_prev
    x = np.zeros(n, np.float32)
    x_next = np.float32(0.0)
    for i in range(n - 1, -1, -1):
        x_next = dp[i] - cp[i] * x_next
        x[i] = x_next
    return x


def spline_quadrant(optim_param):
    """q[i,j] = natural-cubic-spline(mp_log) at r=sqrt((i+.5)^2+(j+.5)^2), [1024,1024]."""
    p = np.asarray(optim_param, np.float32)
    mp = np.repeat(p, UP)
    y = np.concatenate([mp, np.zeros(V // 2, np.float32)])       # len 2048
    n = y.shape[0]
    rhs = (6.0 * (y[2:].astype(np.float64) - 2.0 * y[1:-1] + y[:-2])).astype(np.float32)
    M = np.concatenate([np.zeros(1, np.float32), _thomas(rhs), np.zeros(1, np.float32)])
    half = V // 2
    coord = np.arange(half, dtype=np.float32) + 0.5
    r = np.sqrt(coord[:, None] ** 2 + coord[None, :] ** 2)
    ind = np.clip(np.floor(r).astype(np.int64), 0, n - 2)
    t = r - ind.astype(np.float32)
    y0, y1 = y[ind], y[ind + 1]
    m0, m1 = M[ind], M[ind + 1]
    b = (y1 - y0) - (2.0 * m0 + m1) / 6.0
    return y0 + t * (b + t * (m0 / 2.0 + t * (m1 - m0) / 6.0))


def bias_band(optim_param):
    """Full mirrored bias map on the 2048^2 band."""
    q = spline_quadrant(optim_param)
    row = np.concatenate([q[:, ::-1], q], axis=1)
    return np.concatenate([row[::-1, :], row], axis=0)          # [2048, 2048]


def const_phase_band():
    """(input_phase + lens_phase) on the 2048^2 band, f64."""
    coords = (PITCH * (np.arange(N, dtype=np.float32) - N // 2)).astype(np.float32)
    cb = coords[B0:B0 + V].astype(np.float64)
    r2 = cb[:, None] ** 2 + cb[None, :] ** 2
    return np.float64(K_WAVE) * r2 * (1.0 / (2 * D1) - 1.0 / (2 * FOCAL))


def h_spec_planes():
    """ifftshifted transfer function exp(i*H_phase): (re, im) [4096,4096] f64."""
    fx = ((np.arange(1, N + 1, dtype=np.float32) - np.float32(N / 2)) / np.float32(SENSOR_D)).astype(np.float32)
    FY, FX = np.meshgrid(fx, fx, indexing="ij")
    arg = np.maximum((np.float32(1.0 / LAM)) ** 2 - FX.astype(np.float64) ** 2 - FY.astype(np.float64) ** 2, 0.0)
    w1 = np.sqrt(arg).astype(np.float32)
    hp = (np.float32(TWO_PI) * w1 * np.float32(D2)).astype(np.float32).astype(np.float64)
    hre = np.cos(hp)
    him = np.sin(hp)
    hre = np.fft.ifftshift(hre)
    him = np.fft.ifftshift(him)
    return hre, him


# ---------------- DFT stage matrices (complex->real 2x blocks) ---------------
def _c2r_lhsT(E):
    """Complex matrix E [out m, in k] -> real lhsT [2k, 2m] for out=lhsT.T@rhs."""
    m, k = E.shape
    W = np.zeros((2 * k, 2 * m), np.float64)
    W[:k, :m] = E.real.T
    W[k:, :m] = -E.imag.T
    W[:k, m:] = E.imag.T
    W[k:, m:] = E.real.T
    return W


def stage_a_fwd_mats():
    """WA[c_lo]: [64, 128]; contracts c_hi' (32 band-high-digits), out k_lo."""
    klo = np.arange(64)[:, None]
    chi = np.arange(32)[None, :]
    mats = []
    for c_lo in range(64):
        E = np.exp(-2j * np.pi * ((16 + chi) * klo % 64) / 64.0) \
            * np.exp(-2j * np.pi * (c_lo * klo) / 4096.0)
        mats.append(_c2r_lhsT(E))
    return np.stack(mats).astype(F16)                            # [64, 64, 128]


def stage_b_fwd_mat():
    """WB: [128, 128]; contracts c_lo (64), out k_hi. DFT-64 scaled 1/64."""
    khi = np.arange(64)[:, None]
    clo = np.arange(64)[None, :]
    E = np.exp(-2j * np.pi * (clo * khi % 64) / 64.0) / 64.0
    return _c2r_lhsT(E).astype(F16)                              # [128, 128]


def stage_a_inv_mats(scale):
    """WAI[m_lo]: [128, 128]; contracts m_hi (full 64), out r_lo, +sign, *scale."""
    rlo = np.arange(64)[:, None]
    mhi = np.arange(64)[None, :]
    mats = []
    for m_lo in range(64):
        E = np.exp(2j * np.pi * (mhi * rlo % 64) / 64.0) \
            * np.exp(2j * np.pi * (m_lo * rlo) / 4096.0) * scale
        mats.append(_c2r_lhsT(E))
    return np.stack(mats).astype(F16)                            # [64, 128, 128]


def stage_b_inv_mat():
    """WBI: [128, 64]; contracts m_lo, out r_hi in {26..37}.
    Output partitions: re at 0:12, im at 32:44."""
    rhi = np.arange(RHI_LO, RHI_HI + 1)[:, None]
    mlo = np.arange(64)[None, :]
    E = np.exp(2j * np.pi * (mlo * rhi % 64) / 64.0)
    W = _c2r_lhsT(E)                                             # [128, 24]
    out = np.zeros((128, 64), np.float64)
    out[:, 0:12] = W[:, 0:12]
    out[:, 32:44] = W[:, 12:24]
    return out.astype(F16)


# ============================================================================
#                        host-side per-core input builders
# ============================================================================
def _syn_layout(plane, rs):
    """[2048, 2048] f32 -> [128, 32*RPC] fp16: p = 64u + 32*is + c_hi',
    f = v*RPC + r''  (c' = 32u + v + 64*c_hi').  Duplicated on both is rows."""
    out = np.empty((128, 32, RPC), F16)
    x = plane[rs].reshape(RPC, 32, 2, 32)      # r'', c_hi', u, v
    for u in range(2):
        out[64 * u:64 * u + 32] = x[:, :, u, :].transpose(1, 2, 0)
        out[64 * u + 32:64 * u + 64] = x[:, :, u, :].transpose(1, 2, 0)
    return np.ascontiguousarray(out.reshape(128, 32 * RPC))


def _h_layout(top, bot):
    """[4096 m, 512 k_c] -> [128, 32768] fp16: p = m_hi (top) / 64+m_hi (bot),
    f = chunk*(64*KCHUNK) + m_lo*KCHUNK + k_cc."""
    out = np.empty((128, NCHUNK, 64, KCHUNK), F16)
    t = top.reshape(64, 64, NCHUNK, KCHUNK)    # m_hi, m_lo, chunk, k_cc
    b = bot.reshape(64, 64, NCHUNK, KCHUNK)
    out[:64] = t.transpose(0, 2, 1, 3)
    out[64:] = b.transpose(0, 2, 1, 3)
    return np.ascontiguousarray(out.reshape(128, NCHUNK * 64 * KCHUNK))


_CONST_CACHE = {}


def _shared_consts():
    if "c" not in _CONST_CACHE:
        cph = const_phase_band()
        hre, him = h_spec_planes()
        WA = stage_a_fwd_mats()                    # [64, 64, 128] fp16
        WA2 = np.concatenate([WA, WA], axis=1)     # [64, 128, 128]
        _CONST_CACHE["c"] = dict(
            cph=cph, hre=hre.astype(F16), him=him.astype(F16),
            wa=np.ascontiguousarray(WA2),
            wb=stage_b_fwd_mat(),
            wai=stage_a_inv_mats(1.0 / 64.0),
            wbi=stage_b_inv_mat(),
        )
    return _CONST_CACHE["c"]


def build_core_inputs(optim_param):
    C = _shared_consts()
    bias = bias_band(optim_param).astype(np.float64)
    # centered in [-pi, pi) so Sin-activation args stay within +-3pi/2
    ph = (np.mod(C["cph"] + bias + np.pi, TWO_PI) - np.pi).astype(F32)
    in_maps = []
    for c in range(NC):
        rs = slice(c * RPC, (c + 1) * RPC)
        ks = slice(c * CPC, (c + 1) * CPC)
        hre = C["hre"][:, ks].astype(F32)
        him = C["him"][:, ks].astype(F32)
        in_maps.append({
            "wa": C["wa"], "wb": C["wb"], "wai": C["wai"], "wbi": C["wbi"],
            "ph": _syn_layout(ph, rs),
            "h1": _h_layout(hre, him),
            "h2": _h_layout(-him, hre),
        })
    return in_maps


# ============================================================================
#                              bass / tile program
# ============================================================================
_NC_CACHE = {}


def _build_nc():
    import concourse.bacc as bacc
    import concourse.mybir as mybir
    import concourse.tile as tile

    dt = mybir.dt.float32
    dh = mybir.dt.float16
    nc = bacc.Bacc("TRN2", target_bir_lowering=False, debug=False,
                   num_devices=NC)

    D = {}
    def din(name, shape, dtype):
        D[name] = nc.dram_tensor(name, list(shape), dtype,
                                 kind="ExternalInput").ap()
    din("wa", (64, 128, 128), dh); din("wb", (128, 128), dh)
    din("wai", (64, 128, 128), dh); din("wbi", (128, 64), dh)
    din("ph", (128, 32 * RPC), dh)
    din("h1", (128, NCHUNK * 64 * KCHUNK), dh)
    din("h2", (128, NCHUNK * 64 * KCHUNK), dh)
    for nm, shape in (
        ("a1i_re", [V, CPC]), ("a1i_im", [V, CPC]),
        ("a1o_re", [V, CPC]), ("a1o_im", [V, CPC]),
        ("a2i_re", [N, RPC3]), ("a2i_im", [N, RPC3]),
        ("a2o_re", [N, RPC3]), ("a2o_im", [N, RPC3]),
        ("sc10", [128, 64 * NPEN1]), ("sc11", [128, 64 * NPEN1]),
        ("scf0", [128, 64 * KCHUNK]), ("scf1", [128, 64 * KCHUNK]),
        ("scf2", [128, 64 * KCHUNK]), ("scf3", [128, 64 * KCHUNK]),
        ("sci0", [128, 64 * KCHUNK]), ("sci1", [128, 64 * KCHUNK]),
        ("sci2", [128, 64 * KCHUNK]), ("sci3", [128, 64 * KCHUNK]),
        ("sc3", [128, 64 * RPC3]),
    ):
        D[nm] = nc.dram_tensor(nm, shape, dh).ap()
    D["out"] = nc.dram_tensor("out", [RPC3, BAND_W], dt, kind="ExternalOutput").ap()

    with tile.TileContext(nc) as tc:
        _emit(nc, tc, mybir, D)
    nc.compile()
    return nc


def _emit(nc, tc, mybir, D):
    dt = mybir.dt.float32
    dh = mybir.dt.float16
    AF = mybir.ActivationFunctionType
    HALF_PI = float(np.pi / 2)
    tgl = [0]

    def drain(dst, src):
        # gpsimd cannot access PSUM; rotate vector/scalar only
        tgl[0] ^= 1
        if tgl[0]:
            nc.vector.tensor_copy(dst, src)
        else:
            nc.scalar.copy(dst, src)

    with (
        tc.tile_pool(name="c1", bufs=1) as c1,
        tc.tile_pool(name="ps", bufs=4, space="PSUM") as ppool,
        tc.tile_pool(name="ps4", bufs=1, space="PSUM") as pp4,
    ):
        # ---- constants resident all phases ----
        wa_t = c1.tile([128, 64 * 128], dh, tag="wa")
        nc.sync.dma_start(wa_t[:].rearrange("k (m n) -> k m n", n=128),
                          D["wa"].rearrange("m k n -> k m n"))
        wb_t = c1.tile([128, 128], dh, tag="wb")
        nc.sync.dma_start(wb_t[:], D["wb"])
        wai_t = c1.tile([128, 64 * 128], dh, tag="wai")
        nc.sync.dma_start(wai_t[:].rearrange("k (m n) -> k m n", n=128),
                          D["wai"].rearrange("m k n -> k m n"))
        wbi_t = c1.tile([128, 64], dh, tag="wbi")
        nc.sync.dma_start(wbi_t[:], D["wbi"])
        bvec = c1.tile([128, 1], dt, tag="bv")
        nc.vector.memset(bvec[0:32], HALF_PI)
        nc.vector.memset(bvec[32:64], 0.0)
        nc.vector.memset(bvec[64:96], HALF_PI)
        nc.vector.memset(bvec[96:128], 0.0)

        def WA(j, base):
            return wa_t[base:base + 64, j * 128:(j + 1) * 128]

        def WAI(j):
            return wai_t[:, j * 128:(j + 1) * 128]

        # ============================ P1 ============================
        with tc.tile_pool(name="syn", bufs=2) as spool:
            for half in range(2):
                rsl = slice(half * NPEN1, (half + 1) * NPEN1)
                pht = spool.tile([128, 32 * NPEN1], dh, tag="ph")
                nc.sync.dma_start(
                    pht[:].rearrange("p (v r) -> p v r", r=NPEN1),
                    D["ph"].rearrange("p (v r) -> p v r", r=RPC)[:, :, rsl])
                Ft = spool.tile([128, 32 * NPEN1], dh, tag="F")
                nc.scalar.activation(Ft[:], pht[:], AF.Sin, bias=bvec[:])
                Fv = Ft[:].rearrange("p (v r) -> p v r", r=NPEN1)
                # stage A: 64 matmuls N=128, j-major dense drains
                t1 = spool.tile([128, 64 * NPEN1], dh, tag="t1")
                for g in range(16):
                    ps = ppool.tile([128, 512], dt, tag="ps")
                    for q in range(4):
                        j = 4 * g + q
                        u, v = divmod(j, 32)
                        nc.tensor.matmul(ps[:, q * NPEN1:(q + 1) * NPEN1],
                                         WA(j, 64 * u), Fv[64 * u:64 * u + 64, v, :],
                                         start=True, stop=True)
                    drain(t1[:, g * 512:(g + 1) * 512], ps[:])
                # corner turn via DRAM bounce: t1[p=klo+64is, f=clo*128+pen]
                #   -> t2[p=clo+64is, f=klo*128+pen]
                sc = D["sc10"] if half == 0 else D["sc11"]
                nc.gpsimd.dma_start(sc, t1[:])
                t2 = spool.tile([128, 64 * NPEN1], dh, tag="t2")
                scv = sc.rearrange("(i k) (c n) -> i c k n", i=2, n=NPEN1)
                for isim in range(2):
                    nc.sync.dma_start(
                        t2[64 * isim:64 * isim + 64, :]
                        .rearrange("p (k n) -> p k n", n=NPEN1),
                        scv[isim])
                # stage B: rhs viewed pen-major so t3 matches a1i layout
                t2v = t2[:].rearrange("p (k n) -> p n k", n=NPEN1)
                t3 = spool.tile([128, 64 * NPEN1], dh, tag="t3")
                for m in range(16):
                    ps = ppool.tile([128, 512], dt, tag="ps")
                    nc.tensor.matmul(ps[:], wb_t[:], t2v[:, m * 8:(m + 1) * 8, :],
                                     start=True, stop=True)
                    drain(t3[:, m * 512:(m + 1) * 512], ps[:])
                # out: t3 [p=khi+64is, f=pen*64+klo] -> a1i [(s*256+r)*512 + khl*64+klo]
                for isim, nm in ((0, "a1i_re"), (1, "a1i_im")):
                    dvw = D[nm].rearrange("(s r) (khl kl) -> s khl r kl", s=8, khl=8)
                    for s in range(8):
                        eng = nc.sync if s % 2 == 0 else nc.gpsimd
                        eng.dma_start(
                            dvw[s, :, rsl, :],
                            t3[64 * isim + 8 * s:64 * isim + 8 * s + 8, :]
                            .rearrange("p (pen kl) -> p pen kl", kl=64))

        # ============================ A2A 1 ============================
        rg = [list(range(NC))]
        nc.gpsimd.collective_compute("AllToAll", mybir.AluOpType.bypass,
                                     ins=[D["a1i_re"]], outs=[D["a1o_re"]],
                                     replica_groups=rg)
        nc.gpsimd.collective_compute("AllToAll", mybir.AluOpType.bypass,
                                     ins=[D["a1i_im"]], outs=[D["a1o_im"]],
                                     replica_groups=rg)

        # ============================ P2 (pipelined chunks) ============
        with (
            tc.tile_pool(name="lp", bufs=2) as lpool,
            tc.tile_pool(name="t1p", bufs=1) as t1pool,
            tc.tile_pool(name="t2p", bufs=2) as t2pool,
            tc.tile_pool(name="bigB", bufs=3) as bpool,
            tc.tile_pool(name="tkp", bufs=1) as tkpool,
            tc.tile_pool(name="hp", bufs=1) as hpool,
        ):
            SCF = (D["scf0"], D["scf1"], D["scf2"], D["scf3"])
            SCI = (D["sci0"], D["sci1"], D["sci2"], D["sci3"])
            t2tiles = {}

            def passA(chunk):
                """L-load, col-FFT stage A, drains, corner turn -> t2."""
                L = lpool.tile([128, 32 * KCHUNK], dh, tag="L")
                for isim, nm in ((0, "a1o_re"), (1, "a1o_im")):
                    av = D[nm].rearrange("(rh u v) k -> u rh v k", u=2, v=32)
                    for u in range(2):
                        nc.sync.dma_start(
                            L[64 * u + 32 * isim:64 * u + 32 * isim + 32, :]
                            .rearrange("p (v kc) -> p v kc", kc=KCHUNK),
                            av[u, :, :, chunk * KCHUNK:(chunk + 1) * KCHUNK])
                t1 = t1pool.tile([128, 64 * KCHUNK], dh, tag="t1")
                Lv = L[:].rearrange("p (v kc) -> p v kc", kc=KCHUNK)
                for g in range(16):
                    ps = ppool.tile([128, 512], dt, tag="ps")
                    for q in range(4):
                        j = 4 * g + q
                        u, v = divmod(j, 32)
                        nc.tensor.matmul(ps[:, q * KCHUNK:(q + 1) * KCHUNK],
                                         WA(j, 64 * u),
                                         Lv[64 * u:64 * u + 64, v, :],
                                         start=True, stop=True)
                    drain(t1[:, g * 512:(g + 1) * 512], ps[:])
                # turn: t1[p=mlo+64is, f=clo*K+kcc] -> t2[p=clo+64is, f=mlo*K+kcc]
                nc.gpsimd.dma_start(SCF[chunk], t1[:])
                t2 = t2pool.tile([128, 64 * KCHUNK], dh, tag="t2")
                scv = SCF[chunk].rearrange("(i m) (c k) -> i c m k", i=2, k=KCHUNK)
                for isim in range(2):
                    nc.sync.dma_start(
                        t2[64 * isim:64 * isim + 64, :]
                        .rearrange("p (m k) -> p m k", k=KCHUNK),
                        scv[isim])
                t2tiles[chunk] = t2

            def passB(chunk):
                """stage B + H-mult -> TA/TB, col-IFFT A (accumulating),
                turn, pruned col-IFFT B, a2i writes."""
                t2 = t2tiles.pop(chunk)
                h1f = hpool.tile([128, 64 * KCHUNK], dh, tag="h1")
                h2f = hpool.tile([128, 64 * KCHUNK], dh, tag="h2")
                hoff = chunk * 64 * KCHUNK
                nc.gpsimd.dma_start(h1f[:], D["h1"][:, hoff:hoff + 64 * KCHUNK])
                nc.gpsimd.dma_start(h2f[:], D["h2"][:, hoff:hoff + 64 * KCHUNK])
                TA = bpool.tile([128, 64 * KCHUNK], dh, tag="big")
                TB = bpool.tile([128, 64 * KCHUNK], dh, tag="big")
                for g4 in range(4):
                    ps4 = pp4.tile([128, 2048], dt, tag="ps4")
                    for q in range(4):
                        mm = 4 * g4 + q
                        nc.tensor.matmul(ps4[:, q * 512:(q + 1) * 512], wb_t[:],
                                         t2[:, mm * 512:(mm + 1) * 512],
                                         start=True, stop=True)
                    s4 = slice(g4 * 2048, (g4 + 1) * 2048)
                    nc.vector.tensor_mul(TA[0:64, s4], ps4[0:64], h1f[0:64, s4])
                    nc.vector.tensor_mul(TA[64:128, s4], ps4[0:64], h1f[64:128, s4])
                    nc.vector.tensor_mul(TB[0:64, s4], ps4[64:128], h2f[0:64, s4])
                    nc.vector.tensor_mul(TB[64:128, s4], ps4[64:128], h2f[64:128, s4])
                # col-IFFT stage A contracts TA+TB by PSUM accumulation
                ti = bpool.tile([128, 64 * KCHUNK], dh, tag="big")
                for g in range(16):
                    ps = ppool.tile([128, 512], dt, tag="ps")
                    for q in range(4):
                        j = 4 * g + q
                        nc.tensor.matmul(ps[:, q * KCHUNK:(q + 1) * KCHUNK],
                                         WAI(j),
                                         TA[:, j * KCHUNK:(j + 1) * KCHUNK],
                                         start=True, stop=False)
                        nc.tensor.matmul(ps[:, q * KCHUNK:(q + 1) * KCHUNK],
                                         WAI(j),
                                         TB[:, j * KCHUNK:(j + 1) * KCHUNK],
                                         start=False, stop=True)
                    drain(ti[:, g * 512:(g + 1) * 512], ps[:])
                # turn: ti[p=rlo+64is, f=mlo*K+kcc] -> tj[p=mlo+64is, f=rlo*K+kcc]
                nc.gpsimd.dma_start(SCI[chunk], ti[:])
                tj = bpool.tile([128, 64 * KCHUNK], dh, tag="big")
                scv = SCI[chunk].rearrange("(i r) (m k) -> i m r k", i=2, k=KCHUNK)
                for isim in range(2):
                    nc.scalar.dma_start(
                        tj[64 * isim:64 * isim + 64, :]
                        .rearrange("p (r k) -> p r k", k=KCHUNK),
                        scv[isim])
                # col-IFFT stage B (pruned to 24 rows); rhs viewed kcc-major
                tjv = tj[:].rearrange("p (r k) -> p k r", k=KCHUNK)
                tk = tkpool.tile([64, 64 * KCHUNK], dh, tag="tk")
                for m in range(16):
                    ps = ppool.tile([128, 512], dt, tag="ps")
                    nc.tensor.matmul(ps[0:64, :], wbi_t[:],
                                     tjv[:, m * 8:(m + 1) * 8, :],
                                     start=True, stop=True)
                    drain(tk[0:12, m * 512:(m + 1) * 512], ps[0:12, :])
                    drain(tk[32:44, m * 512:(m + 1) * 512], ps[32:44, :])
                # out: tk [p=idx+32isim, f=kcc*64+rl] -> a2i [(s*512+kc)*96 + idx*8+rl3]
                for isim, nm in ((0, "a2i_re"), (1, "a2i_im")):
                    dvw = D[nm].rearrange("(s kc) (idx rl3) -> s idx kc rl3",
                                          s=8, idx=12)
                    tv = tk[32 * isim:32 * isim + 12, :].rearrange(
                        "p (kc s rl3) -> p kc s rl3", s=8, rl3=8)
                    for s in range(8):
                        nc.gpsimd.dma_start(
                            dvw[s, :, chunk * KCHUNK:(chunk + 1) * KCHUNK, :],
                            tv[:, :, s, :])

            # pipeline: A0 A1 B0 A2 B1 A3 B2 B3 — chunk c+1/c+2 front work
            # overlaps chunk c's back half across engine queues
            passA(0)
            passA(1)
            for chunk in range(NCHUNK):
                passB(chunk)
                if chunk + 2 < NCHUNK:
                    passA(chunk + 2)

        # ============================ A2A 2 ============================
        nc.gpsimd.collective_compute("AllToAll", mybir.AluOpType.bypass,
                                     ins=[D["a2i_re"]], outs=[D["a2o_re"]],
                                     replica_groups=rg)
        nc.gpsimd.collective_compute("AllToAll", mybir.AluOpType.bypass,
                                     ins=[D["a2i_im"]], outs=[D["a2o_im"]],
                                     replica_groups=rg)

        # ============================ P3 ============================
        with (
            tc.tile_pool(name="lp3", bufs=1) as lp3,
            tc.tile_pool(name="to3", bufs=2) as to3,
        ):
            LB = lp3.tile([128, 64 * RPC3], dh, tag="LB")
            for isim, nm in ((0, "a2o_re"), (1, "a2o_im")):
                nc.sync.dma_start(
                    LB[64 * isim:64 * isim + 64, :]
                    .rearrange("p (kl r) -> p kl r", r=RPC3),
                    D[nm].rearrange("(kh kl) r -> kh kl r", kl=64))
            # stage A: contract k_hi, mats by k_lo, out digit c_lo; j-major drains
            t1 = lp3.tile([128, 64 * RPC3], dh, tag="t13")
            for g in range(13):
                ps = ppool.tile([128, 512], dt, tag="ps")
                qn = min(5, 64 - 5 * g)
                for q in range(qn):
                    j = 5 * g + q
                    nc.tensor.matmul(ps[:, q * RPC3:(q + 1) * RPC3],
                                     WAI(j),
                                     LB[:, j * RPC3:(j + 1) * RPC3],
                                     start=True, stop=True)
                drain(t1[:, 5 * g * RPC3:(5 * g + qn) * RPC3],
                      ps[:, 0:qn * RPC3])
            # turn: t1[p=rlo+64is, f=klo*96+r] -> t2[p=klo+64is, f=rlo*96+r]
            nc.gpsimd.dma_start(D["sc3"], t1[:])
            t2 = lp3.tile([128, 64 * RPC3], dh, tag="t23")
            scv = D["sc3"].rearrange("(i c) (k r) -> i k c r", i=2, r=RPC3)
            for isim in range(2):
                nc.sync.dma_start(
                    t2[64 * isim:64 * isim + 64, :]
                    .rearrange("p (c r) -> p c r", r=RPC3),
                    scv[isim])
            # stage B pruned + |.|^2 ; rhs viewed r-major
            t2v = t2[:].rearrange("p (c r) -> p r c", r=RPC3)
            tout = lp3.tile([12, 64 * RPC3], dt, tag="tout")
            for m in range((64 * RPC3) // 512):
                ps = ppool.tile([128, 512], dt, tag="ps")
                nc.tensor.matmul(ps[0:64, :], wbi_t[:],
                                 t2v[:, m * 8:(m + 1) * 8, :],
                                 start=True, stop=True)
                sq1 = to3.tile([12, 512], dt, tag="sq1")
                sq2 = to3.tile([12, 512], dt, tag="sq2")
                nc.scalar.activation(sq1[:], ps[0:12, :], AF.Square)
                nc.scalar.activation(sq2[:], ps[32:44, :], AF.Square)
                nc.vector.tensor_add(tout[:, m * 512:(m + 1) * 512],
                                     sq1[:], sq2[:])
            nc.sync.dma_start(
                D["out"].rearrange("r (ci cl) -> ci r cl", ci=12),
                tout[:].rearrange("p (r cl) -> p r cl", cl=64))


# ============================================================================
#                                   kernel()
# ============================================================================
def kernel(optim_param, _trace=False):
    from concourse.bass_utils import run_bass_kernel_spmd
    if "nc" not in _NC_CACHE:
        _NC_CACHE["nc"] = _build_nc()
    nc = _NC_CACHE["nc"]
    in_maps = build_core_inputs(np.asarray(optim_param, F32))
    res = run_bass_kernel_spmd(nc, in_maps, list(range(NC)), trace=_trace)
    outs = [res.results[c]["out"] for c in range(NC)]      # [96, 768] each
    band = np.empty((BAND_W, BAND_W), np.float64)
    for c in range(NC):
        o = np.asarray(outs[c], np.float64)                # rows idx*8+rl3
        for idx in range(NSEL):
            band[64 * idx + 8 * c:64 * idx + 8 * c + 8, :] = o[8 * idx:8 * idx + 8, :]
    region = band[CROP_OFF:CROP_OFF + WCROP, CROP_OFF:CROP_OFF + WCROP]
    out = (region / region.sum()).astype(F32)[None, None]
    if _trace:
        return out, res
    return out
